# revision 11
# baseline (speedup 1.0000x reference)
"""Transformer decoder block (self-attn + cross-attn + FFN, post-LN) on 8
Trainium2 NeuronCores.

Sharding: zero-collective-free data parallel. 8 cores = 2 batches x 4
query-chunks of 512 tokens. Each core projects K/V for its own 512 tokens
and AllGathers them per batch group (K^T first so its collective starts
~15us earlier); attention runs for its 512 queries over all keys; FFN and
LayerNorms are per-token. Host splits inputs / concats outputs.

On-chip layouts (per 2-head pair):
  - Projections produce Q^T/K^T as [feature, token]; scores are computed
    transposed (S^T = [key, query]) with 2-head row packing (tile_position
    row groups 0/64 run concurrently); exp on the scalar engine straight out
    of PSUM; the causal mask is a multiplicative bf16 input on the vector
    engine.
  - V is [token, feature]; AV contracts keys on partitions with 2-head
    column packing; per-pair AV blocks are issued one 4-tile block behind
    scores so the tensor engine never waits on exp (keeps HAM at 2.4GHz).
  - Softmax denominators: P~ tiles accumulated on the vector engine (bf16),
    one M=1 ones-matmul per head sums partitions, reciprocal, then 1/Z is
    broadcast across the 64 dh partitions with a DRAM-bounce stride-0 DMA
    instead of K=1 PE matmuls.
  - Residual + LayerNorm run per token-tile in [token, feature]
    (bn_stats/bn_aggr) immediately followed by that tile's PE transposes so
    the next block's projections start as early as possible.
  - FFN2 runs kb-major with all 8 PSUM banks holding the 4x2 token/feature
    output regions while W2 streams through SBUF once.
All matmuls bf16 with fp32 PSUM accumulation; residual/LN paths fp32.
"""

from contextlib import ExitStack

import numpy as np
import ml_dtypes

import concourse.bass as bass
import concourse.bacc as bacc
import concourse.mybir as mybir
import concourse.tile as tile
from concourse import bass_utils
from concourse.masks import make_identity

BF16 = mybir.dt.bfloat16
F32 = mybir.dt.float32
AF = mybir.ActivationFunctionType
OP = mybir.AluOpType

B, S, D, H, F = 2, 2048, 1024, 16, 4096
DH = 64
EPS = 1e-5
CH = 512          # tokens per core
DT = D // 128     # 8 feature tiles
NKT = S // 128    # 16 key tiles
NPAIR = H // 2    # 8 head pairs
NMT = CH // 128   # 4 token tiles per core
NFT = F // 128    # 32 FFN hidden tiles

_CACHED = None


def build():
    nc = bacc.Bacc("TRN2", target_bir_lowering=False, debug=False,
                   enable_asserts=False, num_devices=8)

    # ---- per-core DRAM I/O ----
    d_xTq = nc.dram_tensor("xTq", [D, CH], BF16, kind="ExternalInput")
    d_eTq = nc.dram_tensor("eTq", [D, CH], BF16, kind="ExternalInput")
    d_res1 = nc.dram_tensor("res1", [CH, D], F32, kind="ExternalInput")
    d_maskT = nc.dram_tensor("maskT", [S, CH], BF16, kind="ExternalInput")
    wnames = ["sa_wq", "sa_wk", "sa_wv", "sa_wo", "ca_wq", "ca_wk", "ca_wv",
              "ca_wo"]
    d_w = {n: nc.dram_tensor(n, [D, D], BF16, kind="ExternalInput")
           for n in wnames}
    d_w1 = nc.dram_tensor("f_w1", [D, F], BF16, kind="ExternalInput")
    d_w2 = nc.dram_tensor("f_w2", [F, D], BF16, kind="ExternalInput")
    d_bq_sa = nc.dram_tensor("sa_bq", [D], F32, kind="ExternalInput")
    d_bk_sa = nc.dram_tensor("sa_bk", [D], F32, kind="ExternalInput")
    d_bq_ca = nc.dram_tensor("ca_bq", [D], F32, kind="ExternalInput")
    d_bk_ca = nc.dram_tensor("ca_bk", [D], F32, kind="ExternalInput")
    d_b1 = nc.dram_tensor("f_b1", [F], F32, kind="ExternalInput")
    d_cvec = nc.dram_tensor("cvec", [D], BF16, kind="ExternalInput")
    d_b2v = nc.dram_tensor("b2v", [D], BF16, kind="ExternalInput")
    d_gbt = {n: nc.dram_tensor(n, [D], BF16, kind="ExternalInput")
             for n in ["sa_g", "sa_bt", "ca_g", "ca_bt", "f_g", "f_bt"]}
    d_out = nc.dram_tensor("out", [CH, D], F32, kind="ExternalOutput")
    d_rzscr = {p: nc.dram_tensor(f"rzscr_{p}", [16, CH], F32, kind="Internal")
               for p in ("sa", "ca")}
    cc = {}
    for pfx in ("sa", "ca"):
        cc[f"{pfx}_kt_in"] = nc.dram_tensor(f"cc_{pfx}_kt_in", [D, CH], BF16,
                                            kind="Internal")
        cc[f"{pfx}_kt_out"] = nc.dram_tensor(f"cc_{pfx}_kt_out", [4 * D, CH],
                                             BF16, kind="Internal")
        cc[f"{pfx}_v_in"] = nc.dram_tensor(f"cc_{pfx}_v_in", [CH, D], BF16,
                                           kind="Internal")
        cc[f"{pfx}_v_out"] = nc.dram_tensor(f"cc_{pfx}_v_out", [S, D], BF16,
                                            kind="Internal")
    GROUPS = [[0, 1, 2, 3], [4, 5, 6, 7]]

    with tile.TileContext(nc) as tc, ExitStack() as ctx:
        const = ctx.enter_context(tc.tile_pool(name="const", bufs=1))
        qpool = ctx.enter_context(tc.tile_pool(name="qpool", bufs=16))
        resp = ctx.enter_context(tc.tile_pool(name="resp", bufs=8))
        scrp = ctx.enter_context(tc.tile_pool(name="scrp", bufs=2))
        ps_s = ctx.enter_context(tc.tile_pool(name="ps_s", bufs=2,
                                              space="PSUM"))
        ps_av = ctx.enter_context(tc.tile_pool(name="ps_av", bufs=2,
                                               space="PSUM"))
        ps_m = ctx.enter_context(tc.tile_pool(name="ps_m", bufs=2,
                                              space="PSUM"))
        # attention-phase pools, released before the FFN section
        attn_ctx = ExitStack()
        wpool = attn_ctx.enter_context(tc.tile_pool(name="wpool", bufs=16))
        maskp = attn_ctx.enter_context(tc.tile_pool(name="maskp", bufs=2))
        kvp = attn_ctx.enter_context(tc.tile_pool(name="kvp", bufs=2))
        vpp = attn_ctx.enter_context(tc.tile_pool(name="vpp", bufs=2))
        ppool = attn_ctx.enter_context(tc.tile_pool(name="ppool", bufs=10))
        zpool = attn_ctx.enter_context(tc.tile_pool(name="zpool", bufs=2))
        attp = attn_ctx.enter_context(tc.tile_pool(name="attp", bufs=8))
        zsm = attn_ctx.enter_context(tc.tile_pool(name="zsm", bufs=2))
        pools = (attp, kvp, vpp, ppool, zpool, zsm)

        # ---- critical-path input DMAs first ----
        xq = []
        for k in range(DT):
            t = qpool.tile([128, CH], BF16, tag="qt", name=f"xq{k}")
            nc.sync.dma_start(out=t, in_=d_xTq.ap()[k * 128:(k + 1) * 128, :])
            xq.append(t)

        def load_w8(wd, ncols=D):
            ws = []
            for k in range(DT):
                t = wpool.tile([128, ncols], BF16, tag="w", name=f"w_{k}")
                nc.sync.dma_start(out=t, in_=wd.ap()[k * 128:(k + 1) * 128, :])
                ws.append(t)
            return ws

        wk_sa = load_w8(d_w["sa_wk"])

        ident = const.tile([128, 128], F32, tag="ident")
        make_identity(nc, ident)
        onescol = const.tile([128, 1], BF16, tag="onescol")
        nc.vector.memset(onescol, 1.0)
        epst = const.tile([128, 1], F32, tag="epst")
        nc.vector.memset(epst, EPS)
        zerot = const.tile([128, 1], F32, tag="zerot")
        nc.vector.memset(zerot, 0.0)

        def bias_cols(dram, ntiles, name):
            t = const.tile([128, ntiles], F32, tag=name, name=name)
            src = bass.AP(tensor=dram.ap().tensor, offset=0,
                          ap=[[1, 128], [128, ntiles]])
            nc.sync.dma_start(out=t, in_=src)
            return t

        def bcast_row(dram, tag, name):
            t = const.tile([128, D], BF16, tag=tag, bufs=2, name=name)
            src = bass.AP(tensor=dram.ap().tensor, offset=0,
                          ap=[[0, 128], [1, D]])
            nc.sync.dma_start(out=t, in_=src)
            return t

        bq_sa = bias_cols(d_bq_sa, DT, "bqsa")
        bk_sa = bias_cols(d_bk_sa, DT, "bksa")
        bq_ca = bias_cols(d_bq_ca, DT, "bqca")
        bk_ca = bias_cols(d_bk_ca, DT, "bkca")

        def layer_norm(src, g_t, bt_t, out):
            """[128, D] f32 LN along free dim; out may alias src."""
            stats = scrp.tile([128, 2, 6], F32, tag="lnstat", bufs=3,
                              name="lnstat")
            for s in range(2):
                nc.vector.bn_stats(out=stats[:, s, :],
                                   in_=src[:, s * 512:(s + 1) * 512])
            mv = scrp.tile([128, 2], F32, tag="lnmv", bufs=3, name="lnmv")
            nc.vector.bn_aggr(out=mv, in_=stats)
            rstd = scrp.tile([128, 1], F32, tag="lnrstd", bufs=3, name="lnrstd")
            nc.scalar.activation(out=rstd, in_=mv[:, 1:2], func=AF.Sqrt,
                                 bias=epst, scale=1.0)
            nc.vector.reciprocal(out=rstd, in_=rstd)
            cent = scrp.tile([128, D], F32, tag="cent", bufs=2, name="cent")
            nc.vector.scalar_tensor_tensor(out=cent, in0=src, scalar=mv[:, 0:1],
                                           in1=g_t, op0=OP.subtract,
                                           op1=OP.mult)
            nc.vector.scalar_tensor_tensor(out=out, in0=cent, scalar=rstd,
                                           in1=bt_t, op0=OP.mult, op1=OP.add)

        def projT(ws, src_tiles, bias_col, out_tag):
            """out^T [feature, token] tiles: lhsT=weight cols, rhs=src^T."""
            outs = []
            for m in range(DT):
                pool = ps_m if m % 2 == 0 else ps_av
                ps = pool.tile([128, CH], F32, tag=pool.name, name="projps")
                for k in range(DT):
                    nc.tensor.matmul(ps, ws[k][:, m * 128:(m + 1) * 128],
                                     src_tiles[k], start=(k == 0),
                                     stop=(k == DT - 1))
                o = qpool.tile([128, CH], BF16, tag="qt", name=f"{out_tag}{m}")
                nc.scalar.activation(out=o, in_=ps, func=AF.Identity,
                                     bias=bias_col[:, m:m + 1], scale=1.0)
                outs.append(o)
            return outs

        def kv_local_and_ag(pfx, wk, d_wv, bk_col, src_tiles):
            """Project this chunk's K^T/V, stage to DRAM, AllGather ASAP."""
            with nc.named_scope(f"{pfx}_kvlocal"):
                for m in range(DT):
                    pool = ps_m if m % 2 == 0 else ps_av
                    ps = pool.tile([128, CH], F32, tag=pool.name, name="lkps")
                    for k in range(DT):
                        nc.tensor.matmul(ps, wk[k][:, m * 128:(m + 1) * 128],
                                         src_tiles[k], start=(k == 0),
                                         stop=(k == DT - 1))
                    st = scrp.tile([128, CH], BF16, tag="stage", bufs=4,
                                   name="ktst")
                    nc.scalar.activation(out=st, in_=ps, func=AF.Identity,
                                         bias=bk_col[:, m:m + 1], scale=1.0)
                    nc.sync.dma_start(
                        out=cc[f"{pfx}_kt_in"].ap()[m * 128:(m + 1) * 128, :],
                        in_=st)
                nc.gpsimd.collective_compute(
                    "AllGather", mybir.AluOpType.bypass,
                    ins=[cc[f"{pfx}_kt_in"].ap()],
                    outs=[cc[f"{pfx}_kt_out"].ap()],
                    replica_groups=GROUPS)
                wv = load_w8(d_wv)
                for tt in range(NMT):
                    for n in range(2):
                        pool = ps_m if n == 0 else ps_av
                        ps = pool.tile([128, CH], F32, tag=pool.name,
                                       name="lvps")
                        for k in range(DT):
                            nc.tensor.matmul(
                                ps, src_tiles[k][:, tt * 128:(tt + 1) * 128],
                                wv[k][:, n * 512:(n + 1) * 512],
                                start=(k == 0), stop=(k == DT - 1))
                        st = scrp.tile([128, CH], BF16, tag="stage", bufs=4,
                                       name="vst")
                        nc.scalar.activation(out=st, in_=ps, func=AF.Copy)
                        nc.sync.dma_start(
                            out=cc[f"{pfx}_v_in"].ap()[
                                tt * 128:(tt + 1) * 128,
                                n * 512:(n + 1) * 512],
                            in_=st)
                nc.gpsimd.collective_compute(
                    "AllGather", mybir.AluOpType.bypass,
                    ins=[cc[f"{pfx}_v_in"].ap()],
                    outs=[cc[f"{pfx}_v_out"].ap()],
                    replica_groups=GROUPS)

        def attention(pfx, d_ktout, d_vout, QT, masks, pools):
            """masks: callable g -> [128, 2, CH] view or None."""
            attp, kvp, vpp, ppool, zpool, zsm = pools
            BLK = 4
            blocks = [list(range(i, min(i + BLK, NKT)))
                      for i in range(0, NKT, BLK)]
            aun = []
            for hp in range(NPAIR):
                with nc.named_scope(f"{pfx}_pair{hp}"):
                    ktp = kvp.tile([128, 4 * CH], BF16, tag="ktp", name="ktp")
                    nc.sync.dma_start(
                        out=ktp,
                        in_=bass.AP(tensor=d_ktout.ap().tensor,
                                    offset=128 * hp * CH,
                                    ap=[[CH, 128], [D * CH, 4], [1, CH]]))
                    vt = vpp.tile([128, NKT, 128], BF16, tag="vpp", name="vpp")
                    nc.sync.dma_start(
                        out=vt,
                        in_=bass.AP(tensor=d_vout.ap().tensor,
                                    offset=hp * 128,
                                    ap=[[D, 128], [128 * D, NKT], [1, 128]]))
                    pav = ps_av.tile([128, CH], F32, tag="ps_av", name="pav")
                    zacc = zpool.tile([128, 2, CH], BF16, tag="zacc", bufs=2,
                                      name="zacc")
                    pts = {}

                    def issue_scores(blk):
                        for g in blk:
                            ksl = ktp[:, g * 128:(g + 1) * 128]
                            pss = ps_s.tile([128, 2 * CH], F32, tag="ps_s",
                                            name="pss")
                            nc.tensor.matmul(pss[:, 0:CH], ksl[0:64, :],
                                             QT[hp][0:64, :],
                                             start=True, stop=True)
                            nc.tensor.matmul(pss[:, CH:2 * CH], ksl[64:128, :],
                                             QT[hp][64:128, :],
                                             start=True, stop=True)
                            pt = ppool.tile([128, 2, CH], BF16, tag="pt",
                                            bufs=10, name="pt")
                            pt2 = pt.rearrange("p h q -> p (h q)")
                            nc.scalar.activation(out=pt2, in_=pss, func=AF.Exp,
                                                 bias=zerot,
                                                 scale=1.0 / np.sqrt(DH))
                            if masks is not None:
                                nc.vector.tensor_mul(pt, pt, masks(g))
                            if g == 0:
                                nc.vector.tensor_copy(zacc, pt)
                            else:
                                nc.vector.tensor_add(zacc, zacc, pt)
                            pts[g] = pt

                    def issue_av(blk):
                        for g in blk:
                            pt = pts.pop(g)
                            for h in range(2):
                                vsl = vt[:, g, h * 64:(h + 1) * 64]
                                nc.tensor.matmul(
                                    pav[h * 64:(h + 1) * 64, :],
                                    vsl, pt[:, h, :],
                                    start=(g == 0), stop=(g == NKT - 1))

                    for bi, blk in enumerate(blocks):
                        issue_scores(blk)
                        if bi > 0:
                            issue_av(blocks[bi - 1])
                    issue_av(blocks[-1])

                    at = attp.tile([128, CH], BF16, tag="aun", bufs=8,
                                   name=f"aun{hp}")
                    nc.vector.tensor_copy(at, pav)
                    aun.append(at)
                    # per-pair Z: partition-sum, reciprocal, DRAM bounce
                    zacc2 = zacc.rearrange("p h q -> p (h q)")
                    zpair = zsm.tile([2, CH], F32, tag="zpair", bufs=2,
                                     name="zpair")
                    for h in range(2):
                        zf = ps_m.tile([1, CH], F32, tag=ps_m.name, name="zf")
                        nc.tensor.matmul(zf, onescol,
                                         zacc2[:, h * CH:(h + 1) * CH],
                                         start=True, stop=True)
                        zrow = zsm.tile([1, CH], F32, tag="zrow", bufs=2,
                                        name="zrow")
                        nc.vector.reciprocal(out=zrow, in_=zf)
                        nc.sync.dma_start(out=zpair[h:h + 1, :], in_=zrow)
                    nc.sync.dma_start(
                        out=d_rzscr[pfx].ap()[2 * hp:2 * hp + 2, :],
                        in_=zpair)
                    prz = zsm.tile([128, CH], F32, tag="prz", bufs=2,
                                   name="prz")
                    for h in range(2):
                        nc.sync.dma_start(
                            out=prz[h * 64:(h + 1) * 64, :],
                            in_=bass.AP(tensor=d_rzscr[pfx].ap().tensor,
                                        offset=(2 * hp + h) * CH,
                                        ap=[[0, 64], [1, CH]]))
                    nc.vector.tensor_mul(aun[hp], aun[hp], prz)
            return aun

        def wo_resid_ln_T(attnT, d_wo, resid_fn, extra_vec, g_t, bt_t, tag,
                          out_tag):
            """WO + residual + LN per token tile, transposes inline."""
            wo = load_w8(d_wo)
            outs = []
            outsT = [qpool.tile([128, CH], BF16, tag="qt",
                                name=f"{out_tag}{i}") for i in range(DT)]
            for mt in range(NMT):
                pre = resp.tile([128, D], F32, tag="persist", name=f"{tag}{mt}")
                rt = resid_fn(mt)
                for n in range(2):
                    pool = ps_m if n == 0 else ps_av
                    ps = pool.tile([128, 512], F32, tag=pool.name, name="wops")
                    for k in range(DT):
                        nc.tensor.matmul(
                            ps, attnT[k][:, mt * 128:(mt + 1) * 128],
                            wo[k][:, n * 512:(n + 1) * 512],
                            start=(k == 0), stop=(k == DT - 1))
                    nc.vector.tensor_add(pre[:, n * 512:(n + 1) * 512], ps,
                                         rt[:, n * 512:(n + 1) * 512])
                if extra_vec is not None:
                    nc.vector.tensor_add(pre, pre, extra_vec)
                layer_norm(pre, g_t, bt_t, pre)
                for ft in range(DT):
                    pool = ps_m if ft % 2 == 0 else ps_av
                    pst = pool.tile([128, 128], F32, tag=pool.name, name="tps")
                    nc.tensor.transpose(
                        pst, pre[:, ft * 128:(ft + 1) * 128], ident)
                    nc.vector.tensor_copy(
                        outsT[ft][:, mt * 128:(mt + 1) * 128], pst)
                outs.append(pre)
            return outs, outsT

        # ======== attention phases (pools released before FFN) ========
        if True:
            # local K/V + AllGather for both attentions, issued up front
            kv_local_and_ag("sa", wk_sa, d_w["sa_wv"], bk_sa, xq)
            eq = []
            for k in range(DT):
                t = qpool.tile([128, CH], BF16, tag="qt", name=f"eq{k}")
                nc.sync.dma_start(out=t,
                                  in_=d_eTq.ap()[k * 128:(k + 1) * 128, :])
                eq.append(t)
            wk_ca = load_w8(d_w["ca_wk"])
            kv_local_and_ag("ca", wk_ca, d_w["ca_wv"], bk_ca, eq)

            mbs = []
            for b in range(2):
                mb = maskp.tile([128, 8, CH], BF16, tag="mask", bufs=2,
                                name=f"maskb{b}")
                nc.sync.dma_start(
                    out=mb,
                    in_=bass.AP(tensor=d_maskT.ap().tensor,
                                offset=b * 8 * 128 * CH,
                                ap=[[CH, 128], [128 * CH, 8], [1, CH]]))
                mbs.append(mb)

            def masks(g):
                # [128, 2, CH] broadcast view (same mask for both heads)
                mb = mbs[g // 8][:, g % 8, :]
                return mb[:, None, :].broadcast_to([128, 2, CH])

            # ---- self attention ----
            with nc.named_scope("sa_q"):
                wq = load_w8(d_w["sa_wq"])
                QTsa = projT(wq, xq, bq_sa, "qsa")
            attnT = attention("sa", cc["sa_kt_out"], cc["sa_v_out"],
                              QTsa, masks, pools)

            def sa_resid(mt):
                t = scrp.tile([128, D], F32, tag="res1", name="res1t")
                nc.sync.dma_start(
                    out=t, in_=d_res1.ap()[mt * 128:(mt + 1) * 128, :])
                return t

            with nc.named_scope("sa_wo_ln"):
                g1 = bcast_row(d_gbt["sa_g"], "gt", "g1")
                bt1 = bcast_row(d_gbt["sa_bt"], "gt", "bt1")
                x1, x1T = wo_resid_ln_T(attnT, d_w["sa_wo"], sa_resid, None,
                                        g1, bt1, "x1_", "x1T")

            # ---- cross attention ----
            with nc.named_scope("ca_q"):
                wqc = load_w8(d_w["ca_wq"])
                QTca = projT(wqc, x1T, bq_ca, "qca")
            attnTc = attention("ca", cc["ca_kt_out"], cc["ca_v_out"],
                               QTca, None, pools)
            with nc.named_scope("ca_wo_ln"):
                cvec_t = bcast_row(d_cvec, "vec", "cvec")
                g2 = bcast_row(d_gbt["ca_g"], "gt", "g2")
                bt2 = bcast_row(d_gbt["ca_bt"], "gt", "bt2")
                y1, y1T = wo_resid_ln_T(attnTc, d_w["ca_wo"],
                                        lambda mt: x1[mt],
                                        cvec_t, g2, bt2, "y1_", "y1T")
            attn_ctx.close()

        # ======== FFN ========
        b1c = bias_cols(d_b1, NFT, "b1c")
        with ExitStack() as ffn_ctx:
            hpool = ffn_ctx.enter_context(tc.tile_pool(name="hpool", bufs=32))
            with ExitStack() as w1_ctx:
                w1pool = w1_ctx.enter_context(
                    tc.tile_pool(name="w1pool", bufs=8))
                with nc.named_scope("ffn1"):
                    w1 = []
                    for k in range(DT):
                        t = w1pool.tile([128, F], BF16, tag="w1",
                                        name=f"w1_{k}")
                        nc.sync.dma_start(
                            out=t, in_=d_w1.ap()[k * 128:(k + 1) * 128, :])
                        w1.append(t)
                    hT = []
                    for m in range(NFT):
                        pool = ps_m if m % 2 == 0 else ps_av
                        ps = pool.tile([128, CH], F32, tag=pool.name,
                                       name="f1ps")
                        for k in range(DT):
                            nc.tensor.matmul(
                                ps, w1[k][:, m * 128:(m + 1) * 128],
                                y1T[k], start=(k == 0), stop=(k == DT - 1))
                        h = hpool.tile([128, CH], BF16, tag="h", name=f"h{m}")
                        nc.scalar.activation(out=h, in_=ps, func=AF.Relu,
                                             bias=b1c[:, m:m + 1], scale=1.0)
                        hT.append(h)
            with ExitStack() as w2_ctx:
                w2pool = w2_ctx.enter_context(
                    tc.tile_pool(name="w2pool", bufs=8))
                with nc.named_scope("ffn2"):
                    b2v_t = bcast_row(d_b2v, "vec", "b2v")
                    # 8 psum regions (mt, n) live across the whole kb loop
                    pss01 = [ps_s.tile([128, 2 * CH], F32, tag="ps_s",
                                       name=f"f2ps{i}") for i in range(2)]
                    psmt = [ps_m.tile([128, CH], F32, tag=ps_m.name,
                                      name=f"f2pm{i}") for i in range(2)]
                    psav = [ps_av.tile([128, CH], F32, tag="ps_av",
                                       name=f"f2pa{i}") for i in range(2)]

                    def f2psum(mt, n):
                        if mt < 2:
                            return pss01[mt][:, n * CH:(n + 1) * CH]
                        return (psmt if mt == 2 else psav)[n]

                    for kb in range(4):
                        w2b = []
                        for n in range(2):
                            t = w2pool.tile([128, 8, 512], BF16, tag="w2",
                                            name=f"w2b{kb}_{n}")
                            nc.sync.dma_start(
                                out=t,
                                in_=bass.AP(
                                    tensor=d_w2.ap().tensor,
                                    offset=kb * 8 * 128 * D + n * 512,
                                    ap=[[D, 128], [128 * D, 8], [1, 512]]))
                            w2b.append(t)
                        for ks in range(8):
                            k = kb * 8 + ks
                            for mt in range(NMT):
                                for n in range(2):
                                    nc.tensor.matmul(
                                        f2psum(mt, n),
                                        hT[k][:, mt * 128:(mt + 1) * 128],
                                        w2b[n][:, ks, :],
                                        start=(k == 0), stop=(k == NFT - 1))
                    with nc.named_scope("ln3_out"):
                        g3 = bcast_row(d_gbt["f_g"], "gt", "g3")
                        bt3 = bcast_row(d_gbt["f_bt"], "gt", "bt3")
                        for mt in range(NMT):
                            h2 = resp.tile([128, D], F32, tag="persist",
                                           name=f"h2_{mt}")
                            for n in range(2):
                                nc.vector.tensor_add(
                                    h2[:, n * CH:(n + 1) * CH],
                                    f2psum(mt, n),
                                    y1[mt][:, n * CH:(n + 1) * CH])
                            nc.vector.tensor_add(h2, h2, b2v_t)
                            layer_norm(h2, g3, bt3, h2)
                            nc.sync.dma_start(
                                out=d_out.ap()[mt * 128:(mt + 1) * 128, :],
                                in_=h2)

    nc.compile()
    return nc


def _bf(a):
    return np.ascontiguousarray(a, dtype=np.float32).astype(ml_dtypes.bfloat16)


def kernel(**inputs):
    global _CACHED
    if _CACHED is None:
        _CACHED = build()
    nc = _CACHED

    f = {k: np.asarray(v, dtype=np.float32) for k, v in inputs.items()}
    dec, enc = f["decoder_input"], f["encoder_output"]
    cvec = (f["ca_bv"] @ f["ca_wo"] + f["ca_bo"]).astype(np.float32)
    r1vec = (f["sa_bv"] @ f["sa_wo"] + f["sa_bo"]).astype(np.float32)

    shared = {n: _bf(f[n]) for n in
              ["sa_wq", "sa_wk", "sa_wv", "sa_wo",
               "ca_wq", "ca_wk", "ca_wv", "ca_wo", "f_w1", "f_w2"]}
    shared.update({n: f[n] for n in
                   ["sa_bq", "sa_bk", "ca_bq", "ca_bk", "f_b1"]})
    shared["cvec"] = _bf(cvec)
    shared["b2v"] = _bf(f["f_b2"])
    for n in ["sa_g", "sa_bt", "ca_g", "ca_bt", "f_g", "f_bt"]:
        shared[n] = _bf(f[n])

    kk = np.arange(S, dtype=np.int64)[:, None]
    in_maps = []
    for c in range(8):
        b, j = c // 4, c % 4
        rows = slice(j * CH, (j + 1) * CH)
        qq = np.arange(j * CH, (j + 1) * CH, dtype=np.int64)[None, :]
        m = {
            "xTq": _bf(dec[b, rows, :].T),
            "eTq": _bf(enc[b, rows, :].T),
            "res1": np.ascontiguousarray(dec[b, rows, :] + r1vec[None, :]),
            "maskT": (kk <= qq).astype(ml_dtypes.bfloat16),
        }
        m.update(shared)
        in_maps.append(m)

    res = bass_utils.run_bass_kernel_spmd(nc, in_maps, core_ids=list(range(8)))
    out = np.empty((B, S, D), dtype=np.float32)
    for c in range(8):
        b, j = c // 4, c % 4
        out[b, j * CH:(j + 1) * CH, :] = res.results[c]["out"]
    return out
